# revision 42
# baseline (speedup 1.0000x reference)
"""Causal self-attention (B=2, T=2048, D=1024, H=16, hd=64) on 8 TRN2 cores.

Sharding: 2 batches x 4 head-groups (4 heads each). Each core computes the
full pipeline for its (batch, head-group): qkv projection (transposed
layout), causal attention, and its partial output projection. The host sums
the 4 per-batch partials (tensor-parallel reduce) and adds bproj.

Device-side layout notes:
 - x is passed pre-transposed (xT [D, T]) so the qkv projection can contract
   over D on the partition dimension.
 - Scores are computed transposed (St = k @ qT, [k_tok, q_tok]) so softmax's
   exp feeds straight into att@v as the moving operand without transposes.
 - St contracts the FULL 128 partitions: stationary [kA;kB], moving the
   zero-padded per-slice q buffers [qA;0] / [0;qB]. 64-contract
   tile_position f32r matmuls run at 2 cyc/col on HW; full-contract runs
   at 1 cyc/col, so the zero-padding halves St time outright.
 - Softmax has no max-subtraction (scores are O(6) here, exp is safe) and the
   denominator is produced by augmenting v with a ones column (M=65 matmul).
 - The 1/sqrt(hd) scale is folded into Wq/bq on the host. The k bias is
   dropped (q.bk is constant along each softmax row -> invariant); the v
   bias is folded into bproj on the host (attention weights sum to 1), so
   k/v PSUM exits are plain ACT copies, off DVE.
 - exp output, v (vnat) and the causal masks are bf16: same engine rates,
   half the SBUF, and ~0.4% relative error (gate is 2e-2).
 - Softmax reciprocal is DVE reciprocal_approx_fast (keeps the recip off
   the ACT exp FIFO, which otherwise starves St PSUM slots at slice
   boundaries); the partition-crossing rec broadcast is 2 small SBUF DMAs
   split across both HWDGE rings.
 - Head: w / xt0 / xt1 DMAs are interleaved per k-tile across BOTH HWDGE
   rings (sync + scalar) and slice-0's qkv runs k-outer so the PE works at
   DMA arrival rate from ~13us instead of idling ~22us behind one FIFO.
 - The PE HAM clock gate (cold = 1.2 GHz, warm = 2.4 GHz after ~3.4us of
   sustained activity) is the reason for all the density games: paced
   filler quota in pump(), atomic qkv fill groups (short PSUM slot holds),
   and scratch warm-keeper matmuls in the final drain while the last
   recip/norm chains run.
 - proj contracts 128 (two heads stacked); odd heads reach partitions 64:128
   of the stacked y via a small SBUF->SBUF DMA (DVE cannot shift partitions).
"""

import sys

sys.path.insert(0, "/opt/trn_rl_repo")

import numpy as np
from collections import deque

B, T, D = 2, 2048, 1024
N_HEAD = 16
HD = 64  # head dim
HPC = 4  # heads per core
N_CORES = 8

P = 128
NJ = 512  # q-slice width
JT = T // NJ  # 4 q-slices
KT = D // P  # 8 contraction tiles for qkv
MT = 6  # qkv m-tiles: 2 q, 2 k, 2 v (128 dims each)
NQKV = MT * P  # 768
IT = T // P  # 16 k-token tiles

_CACHE = {}
DEBUG = False


def _build():
    import concourse.bass as bass  # noqa: F401
    import concourse.mybir as mybir
    from concourse.ap import AP
    import concourse.tile as tile
    from concourse import bacc

    F32 = mybir.dt.float32
    F32R = mybir.dt.float32r
    BF16 = mybir.dt.bfloat16
    AF = mybir.ActivationFunctionType

    class _Bacc(bacc.Bacc):
        def insert_act_table_loads(self):
            # Force every activation to resolve to the one table set that
            # holds Exp so there is a single table load at kernel start.
            from concourse.hw_specs import get_activation_tables
            import bass_rust as _br

            AF2 = mybir.ActivationFunctionType
            has_activation = any(
                isinstance(i, mybir.InstActivation)
                for b in self.main_func.blocks
                for i in b.instructions
            )
            if not has_activation:
                return
            tables = []
            for name, fns in get_activation_tables(self.m.arch).items():
                if name != "natural_log_exp_and_others":
                    fns = fns - {AF2.Exp, AF2.Ln}
                tables.append((name, fns))
            _br.insert_act_table_loads(self, tables)

    nc = _Bacc(None, target_bir_lowering=False)
    xT_d = nc.dram_tensor("xT", [D, T], F32R, kind="ExternalInput")
    wqkv_d = nc.dram_tensor("wqkv", [D, NQKV], F32R, kind="ExternalInput")
    bqkv_d = nc.dram_tensor("bqkv2", [P, MT], F32, kind="ExternalInput")
    wproj_d = nc.dram_tensor("wproj", [P, 2 * D], F32R, kind="ExternalInput")
    masks_d = nc.dram_tensor("masks", [P, P], BF16, kind="ExternalInput")
    ident_d = nc.dram_tensor("ident", [P, P], F32R, kind="ExternalInput")
    out_d = nc.dram_tensor("out", [T, D], F32, kind="ExternalOutput")

    with tile.TileContext(nc) as tc:
        with (
            tc.tile_pool(name="const", bufs=1) as const,
            tc.tile_pool(name="xp", bufs=2) as xp,
            tc.tile_pool(name="qzp", bufs=2) as qzp,
            tc.tile_pool(name="stps", bufs=2, space="PSUM") as stps,
            tc.tile_pool(name="yps", bufs=2, space="PSUM") as yps,
            tc.tile_pool(name="expp", bufs=10) as expp,
            tc.tile_pool(name="recp", bufs=2) as recp,
            tc.tile_pool(name="bcp", bufs=2) as bcp,
            tc.tile_pool(name="outp", bufs=2) as outp,
        ):
            # Per-k weight tiles so a consumer of k-tile k depends only on
            # its own DMA (not the whole-weight load).
            w_k = [
                const.tile([P, NQKV], F32R, tag=f"wk{k}", name=f"wk{k}")
                for k in range(KT)
            ]
            bias_sb = const.tile([P, MT], F32)
            wp_sb = const.tile([P, 2, D], F32R)
            masks_sb = const.tile([P, P], BF16)
            ident = const.tile([P, P], F32R)
            # k (slots 0,1 = head-pairs) and v (slots 2,3) in transposed
            # layout. q lives in per-slice zero-padded buffers (qz below):
            # St contracts the FULL 128 partitions ([kA;kB] stationary
            # against [qA;0] / [0;qB] moving) because 64-contract
            # tile_position matmuls run at half rate on f32r.
            qkvT_sb = const.tile([P, 4, T], F32R)
            vnat_sb = const.tile([P, 2, IT, 192], BF16)
            yt2_sb = const.tile([P, 2, T], F32R)

            w_r = wqkv_d.rearrange("(kt p) n -> p kt n", p=P)
            xT_r = xT_d.rearrange("(kt p) t -> p kt t", p=P)

            xts = {}

            def emit_xt(j, eng=None):
                eng = eng or nc.sync
                tl = []
                for k in range(KT):
                    t = xp.tile([P, NJ], F32R, tag=f"xt{k}", name=f"xt{j}_{k}")
                    eng.dma_start(t[:], xT_r[:, k, j * NJ : (j + 1) * NJ])
                    tl.append(t)
                xts[j] = tl

            # ---- Head: interleaved dual-ring DMAs -----------------------
            # sync ring: xt0 k-tiles then wproj; scalar ring: w k-tiles then
            # bias/masks/ident. Paired k emission means k-tile pairs arrive
            # in ascending-k order at roughly HBM rate on both rings.
            xt0 = []
            for k in range(KT):
                t = xp.tile([P, NJ], F32R, tag=f"xt{k}", name=f"xt0_{k}")
                nc.sync.dma_start(t[:], xT_r[:, k, 0:NJ])
                xt0.append(t)
                nc.scalar.dma_start(w_k[k][:], w_r[:, k, :])
            xts[0] = xt0
            nc.sync.dma_start(bias_sb[:], bqkv_d[:])
            nc.sync.dma_start(masks_sb[:], masks_d[:])
            nc.sync.dma_start(ident[:], ident_d[:])
            # slice-1 x tiles + wproj go on the sync ring: anything queued
            # on the scalar ring blocks the ACT exp stream behind its
            # backpressured DMA issues.
            xt1 = []
            for k in range(KT):
                t = xp.tile([P, NJ], F32R, tag=f"xt{k}", name=f"xt1_{k}")
                nc.sync.dma_start(t[:], xT_r[:, k, NJ : 2 * NJ])
                xt1.append(t)
            xts[1] = xt1
            nc.sync.dma_start(
                wp_sb[:], wproj_d.rearrange("p (g d) -> p g d", g=2)
            )

            for h2 in range(2):
                nc.gpsimd.memset(vnat_sb[:, h2, :, :], 1.0)

            # ---- Slice-0 qkv: k-outer so the PE tracks DMA arrival ------
            # 3 m-pair accumulators (q, k, v pairs): 2 from the st pool, 1
            # borrowed from the y pool (attention has not started yet).
            pair_q = stps.tile([P, 2, NJ], F32, tag="st", name="q0_qpair")
            pair_k = stps.tile([P, 2, NJ], F32, tag="st", name="q0_kpair")
            pair_v = yps.tile([P, 2, NJ], F32, tag="y", name="q0_vpair")
            pairs = [pair_q, pair_k, pair_v]
            for k in range(KT - 1):
                for mp in (1, 0, 2):
                    for g in range(2):
                        m = 2 * mp + g
                        nc.tensor.matmul(
                            pairs[mp][:, g, :],
                            w_k[k][:, m * P : (m + 1) * P],
                            xts[0][k][:],
                            start=(k == 0),
                            stop=False,
                        )

            def _head_last_k(mp):
                for g in range(2):
                    m = 2 * mp + g
                    nc.tensor.matmul(
                        pairs[mp][:, g, :],
                        w_k[KT - 1][:, m * P : (m + 1) * P],
                        xts[0][KT - 1][:],
                        start=False,
                        stop=True,
                    )
            # Per-slice zero-padded q buffers: qz[(j, hp)] holds
            # [qA; 0] (slot 0) and [0; qB] (slot 1) for head-pair hp.
            qz_map = {}

            def alloc_qz(j, hp):
                t = qzp.tile([P, 2, NJ], F32R, tag=f"qz{hp}", name=f"qz{j}_{hp}")
                nc.gpsimd.memset(t[:, :, :].bitcast(F32), 0.0)
                qz_map[(j, hp)] = t
                return t

            def emit_q_adds(j, hp, ps, on_act=False):
                # ps: [P, NJ]-shaped PSUM AP holding m-tile hp of q.
                # Fills run these on ACT (Identity takes a [P,1] bias AP and
                # shares the exp table set): at boundaries ACT is starved
                # waiting for the next St, so the adds clear immediately
                # instead of queueing in the bursty boundary DVE window.
                t = qz_map[(j, hp)]
                nc.vector.tensor_scalar_add(
                    t[0:HD, 0, :], ps[0:HD], bias_sb[0:HD, hp : hp + 1]
                )
                nc.vector.tensor_scalar_add(
                    t[HD:P, 1, :], ps[HD:P], bias_sb[HD:P, hp : hp + 1]
                )

            # exits: k pair first (St needs k tiles), then q, then v.
            # k/v exits are plain copies on ACT (k bias is softmax-
            # invariant; v bias is folded into bproj on the host).
            _head_last_k(1)
            for g in range(2):
                nc.scalar.activation(
                    qkvT_sb[:, g, 0:NJ], pair_k[:, g, :], AF.Copy
                )
            _head_last_k(0)
            for hp in range(2):
                alloc_qz(0, hp)
                emit_q_adds(0, hp, pair_q[:, hp, :])
            _head_last_k(2)
            for g in range(2):
                nc.scalar.activation(
                    qkvT_sb[:, 2 + g, 0:NJ], pair_v[:, g, :], AF.Copy
                )

            # ---- Stage-2 fills for later slices (schedulable units) -----

            def emit_qkv_m(j, m):
                # one atomic fill: 8 accumulating matmuls + the PSUM exit,
                # so the st-pool slot is held for only ~1 unit
                ps = stps.tile([P, NJ], F32, tag="st", name=f"qkvps{j}_{m}")
                if m < 2:
                    alloc_qz(j, m)
                for k in range(KT):
                    nc.tensor.matmul(
                        ps[:],
                        w_k[k][:, m * P : (m + 1) * P],
                        xts[j][k][:],
                        start=(k == 0),
                        stop=(k == KT - 1),
                    )
                if m < 2:
                    emit_q_adds(j, m, ps[:], on_act=True)
                else:
                    nc.scalar.activation(
                        qkvT_sb[:, m - 2, j * NJ : (j + 1) * NJ],
                        ps[:],
                        AF.Copy,
                    )

            def emit_vt(h2, ii):
                pt = stps.tile([P, NJ], F32R, tag="st", name=f"vt{h2}_{ii}")
                nc.tensor.transpose(
                    pt[:, 0:P], qkvT_sb[:, 2 + h2, ii * P : (ii + 1) * P], ident[:]
                )
                nc.vector.tensor_copy(vnat_sb[:, h2, ii, 0:HD], pt[:, 0:HD])
                nc.vector.tensor_copy(
                    vnat_sb[:, h2, ii, 2 * HD : 2 * HD + P - HD], pt[:, HD:P]
                )

            # slice-0 v transposes
            for h2 in range(2):
                for ii in range(4):
                    emit_vt(h2, ii)


            fill_q = deque()

            def push_fill(j):
                for m in range(MT):
                    fill_q.append(("qkv", j, m))
                for h2 in range(2):
                    for ii in range(4 * j, 4 * j + 4):
                        fill_q.append(("vt", j, h2, ii))

            def emit_fill(item):
                if item[0] == "qkv":
                    emit_qkv_m(item[1], item[2])
                else:
                    emit_vt(item[2], item[3])

            # pre-emit a few slice-1 qkv fills: their matmuls cover the PE
            # during slice 0's exp-bound St ramp (xt1 landed in the head)
            push_fill(1)
            for _ in range(3):
                emit_fill(fill_q.popleft())

            # ---- Stage 3: attention per head-pair ----------------------
            # Software-pipelined: St(i) is issued before Y(i-1) so ACT's
            # exp(i-1) overlaps the PE's St(i); normalization of slice (hp,j)
            # is deferred into slice (hp,j)+1's loop so the reciprocal's
            # latency hides behind matmul work.

            def emit_recip(pend):
                # 1/d on DVE (reciprocal_approx_fast, ~18 bits): keeps the
                # reciprocal off the ACT exp stream, whose FIFO backlog was
                # starving the St PSUM slots at every slice boundary. The Y
                # matmuls deliver d pre-broadcast: par0's on partitions
                # 64:128, par1's on 0:64. A small SBUF->SBUF DMA moves each
                # reciprocal to the partition range its y values occupy
                # (engines cannot cross partitions; DMA can).
                hp_, j_, y2_ = pend
                recb = recp.tile([P, 2, NJ], F32, tag="recb")
                # full-tile recip: the y quadrants produce junk in slots the
                # DMAs below never read; one wide DVE op per stage beats two
                # narrow ones on per-op overhead.
                nc.vector.reciprocal_approx_fast(recb[:, :, :], y2_[:, :, :])
                rec2 = bcp.tile([P, 2, NJ], F32, tag="bc")
                nc.sync.dma_start(rec2[0:HD, 0, :], recb[64:128, 0, :])
                nc.scalar.dma_start(rec2[64:128, 1, :], recb[0:HD, 1, :])
                return (hp_, j_, y2_, rec2)

            def emit_norm(pend):
                hp_, j_, y2_, rec2 = pend
                # head 2*hp_ -> partitions 0:64; head 2*hp_+1 -> 64:128
                nc.vector.tensor_mul(
                    yt2_sb[0:HD, hp_, j_ * NJ : (j_ + 1) * NJ],
                    y2_[0:HD, 0, :],
                    rec2[0:HD, 0, :],
                )
                nc.vector.tensor_mul(
                    yt2_sb[HD:P, hp_, j_ * NJ : (j_ + 1) * NJ],
                    y2_[HD:P, 1, :],
                    rec2[HD:P, 1, :],
                )

            def emit_y(rec):
                hp_, pi, pexp, y2_, last, j_ = rec
                r_ = pi - 4 * j_
                # f32r matmuls below 256 moving cols run at 1/4 rate, so
                # clamp the diag-block start column to keep >=256 width; the
                # extra columns read memset zeros from the exp tile.
                cy = min(P * r_, NJ - 2 * P) if r_ > 0 else 0
                # weight layout per k-tile: [vA(0:64) | ones(64:128) | vB
                # (128:192)]. par0 uses cols 0:128 -> yA on partitions 0:64,
                # denominator replicated on 64:128. par1 uses cols 64:192 ->
                # denominator on 0:64, yB on 64:128 (where the stacked proj
                # layout wants it -- no partition shift needed).
                nc.tensor.matmul(
                    y2_[:, 0, cy:NJ],
                    vnat_sb[:, hp_, pi, 0:P],
                    pexp[:, 0, cy:NJ],
                    start=(pi == 0),
                    stop=last,
                )
                nc.tensor.matmul(
                    y2_[:, 1, cy:NJ],
                    vnat_sb[:, hp_, pi, HD : HD + P],
                    pexp[:, 1, cy:NJ],
                    start=(pi == 0),
                    stop=last,
                )

            drain_mode = [False]

            def emit_proj(item):
                qm_i, n = item
                po = stps.tile([P, 2, NJ], F32, tag="st")
                for g in range(2):
                    nc.tensor.matmul(
                        po[:, 0, :],
                        yt2_sb[:, g, qm_i * P : (qm_i + 1) * P],
                        wp_sb[:, g, n * NJ : (n + 1) * NJ],
                        start=(g == 0),
                        stop=(g == 1),
                    )
                ot = outp.tile([P, NJ], F32)
                # in the drain ACT is idle: alternate the PSUM exit between
                # DVE and ACT so proj items don't serialize on one copy engine
                if drain_mode[0] and (qm_i + n) % 2 == 1:
                    nc.scalar.activation(ot[:], po[:, 0, :], AF.Copy)
                else:
                    nc.vector.tensor_copy(ot[:], po[:, 0, :])
                nc.sync.dma_start(
                    out_d[qm_i * P : (qm_i + 1) * P, n * NJ : (n + 1) * NJ], ot[:]
                )

            # Global software pipeline over slices (j outer, hp inner):
            # y matmuls trail the St/exp stream by DEPTH iterations and spill
            # across slice boundaries; each slice's normalization runs inside
            # a later slice's loop; proj groups for q-slice j are spread one
            # per iteration once both head-pairs of j are normalized.
            DEPTH = 6
            y_q = deque()  # (hp, i, exp2, y2holder, last, j)
            norm_q = deque()  # (hp, j, y2, rec)
            proj_items = deque()  # (qm_i, n)
            y2_map = {}  # (hp, j) -> y2 tile, allocated lazily at first y

            def emit_y2(rec):
                hp_, pi, pexp, _, last, j_ = rec
                if pi == 0:
                    y2_map[(hp_, j_)] = yps.tile(
                        [P, 2, NJ], F32, tag="y", name=f"y2_{hp_}_{j_}"
                    )
                emit_y((hp_, pi, pexp, y2_map[(hp_, j_)], last, j_))
                if last:
                    norm_q.append(
                        emit_recip((hp_, j_, y2_map.pop((hp_, j_))))
                    )

            calls_left = [1]

            def pump(i):
                # pace PE filler work (qkv/vt fills + proj) evenly across the
                # slice's pump calls so every ACT-bound unit gets PE cover and
                # the DVE bias-adds land away from the boundary burst
                if i >= 2 and norm_q:
                    pend = norm_q.popleft()
                    emit_norm(pend)
                    if pend[0] == 1:  # second head-pair of slice j done
                        for qq in range(4 * pend[1], 4 * pend[1] + 4):
                            proj_items.append((qq, 0))
                            proj_items.append((qq, 1))
                avail = len(fill_q) + (len(proj_items) if i >= 2 else 0)
                quota = -(-avail // max(1, calls_left[0]))
                for _ in range(min(quota, 2)):
                    if fill_q:
                        emit_fill(fill_q.popleft())
                    elif i >= 2 and proj_items:
                        emit_proj(proj_items.popleft())
                calls_left[0] = max(1, calls_left[0] - 1)

            for j in range(JT):
                if j + 2 < JT:
                    emit_xt(j + 2)
                if 0 < j and j + 1 < JT:
                    push_fill(j + 1)
                n_i = 4 * j + 4
                calls_left[0] = 2 * n_i
                for i in range(n_i):
                    for hp in range(2):
                        if len(y_q) > DEPTH:
                            emit_y2(y_q.popleft())
                        st2 = stps.tile([P, 2, NJ], F32, tag="st")
                        r = i - 4 * j
                        c0 = P * r if r > 0 else 0
                        # St matmuls also keep >=256 moving width (the extra
                        # columns land in the masked region the exp memset
                        # overwrites).
                        cs = min(c0, NJ - 2 * P)
                        # full-128 contraction: stationary [kA;kB], moving
                        # the zero-padded [qA;0] / [0;qB] buffers. One
                        # LDWEIGHTS serves both head outputs.
                        kst = qkvT_sb[:, hp, i * P : (i + 1) * P]
                        qzj = qz_map[(j, hp)]
                        nc.tensor.matmul(
                            st2[:, 0, cs:NJ],
                            kst,
                            qzj[:, 0, cs:NJ],
                            start=True,
                            stop=True,
                        )
                        nc.tensor.matmul(
                            st2[:, 1, cs:NJ],
                            kst,
                            qzj[:, 1, cs:NJ],
                            start=True,
                            stop=True,
                        )
                        exp2 = expp.tile([P, 2, NJ], BF16, tag="exp")
                        if r < 0:
                            nc.scalar.activation(exp2[:], st2[:], AF.Exp)
                        else:
                            # diag block: cols [0, 128r) are fully above the
                            # causal line -> zero; cols [128r, 128r+128) are
                            # triangular; the rest is fully kept.
                            if c0 > 0:
                                nc.vector.memset(
                                    exp2[:, :, 0:c0], 0.0
                                )
                            nc.scalar.activation(
                                exp2[:, :, c0:NJ], st2[:, :, c0:NJ], AF.Exp
                            )
                            m_ap = masks_sb[:, 0:P]
                            m_bc = AP(
                                m_ap.tensor,
                                m_ap.offset,
                                [list(m_ap.ap[0]), [0, 2], list(m_ap.ap[1])],
                            )
                            nc.vector.tensor_mul(
                                exp2[:, :, c0 : c0 + P],
                                exp2[:, :, c0 : c0 + P],
                                m_bc,
                            )
                        y_q.append((hp, i, exp2, None, i == n_i - 1, j))
                        pump(i)
                # slice j+1's qkv must be complete before its St reads;
                # vt items may ride over the boundary (Y consumes the new
                # v tiles only DEPTH units into the next slice, and the
                # tile framework orders the writes regardless)
                while fill_q and fill_q[0][0] == "qkv":
                    emit_fill(fill_q.popleft())

            # drain hp0's Y stream first so its recip/norm/proj overlap
            # hp1's remaining Y matmuls
            for rec in sorted(y_q, key=lambda r: (r[0], r[1])):
                emit_y2(rec)
            y_q.clear()

            def emit_warmkeep(n):
                # scratch matmuls that keep the PE activity monitor hot
                # while the recip/norm chains run, so the final projs
                # execute at full clock instead of half
                for _ in range(n):
                    wk = stps.tile([P, NJ], F32, tag="st", name="warm")
                    nc.tensor.matmul(
                        wk[:], w_k[0][:, 0:P], xts[JT - 1][0][:],
                        start=True, stop=True,
                    )

            drain_mode[0] = True
            emit_warmkeep(3)
            while norm_q:
                pend = norm_q.popleft()
                emit_norm(pend)
                if pend[0] == 1:
                    for qq in range(4 * pend[1], 4 * pend[1] + 4):
                        proj_items.append((qq, 0))
                        proj_items.append((qq, 1))
                emit_warmkeep(3)
            while proj_items:
                emit_proj(proj_items.popleft())

    nc.compile()
    return nc


def _prep_inputs(x, Wqkv, bqkv, Wproj):
    """Per-core input maps. Core c -> batch c//4, heads 4*(c%4) .. +4."""
    import ml_dtypes

    scale = np.float32(1.0 / np.sqrt(HD))
    pp = np.arange(P)[:, None]
    ff = np.arange(P)[None, :]
    masks = (ff >= pp).astype(ml_dtypes.bfloat16)

    in_maps = []
    for c in range(N_CORES):
        b, g = divmod(c, HPC)
        cs = slice(256 * g, 256 * g + 256)
        wq = Wqkv[:, 0 * D :][:, cs] * scale
        wk = Wqkv[:, 1 * D : 2 * D][:, cs]
        wv = Wqkv[:, 2 * D : 3 * D][:, cs]
        wqkv_c = np.ascontiguousarray(np.concatenate([wq, wk, wv], axis=1), np.float32)
        bq = bqkv[0 * D :][cs] * scale
        bk = bqkv[1 * D : 2 * D][cs]
        bv = bqkv[2 * D : 3 * D][cs]
        bqkv_c = np.concatenate([bq, bk, bv]).reshape(MT, P).T
        # head-pairs stacked on partitions: row p, pair g2, col d ->
        # Wproj[256*g + 128*g2 + p, d]
        wproj_c = (
            Wproj[256 * g : 256 * (g + 1), :]
            .reshape(2, P, D)
            .transpose(1, 0, 2)
            .reshape(P, 2 * D)
        )
        in_maps.append(
            {
                "xT": np.ascontiguousarray(x[b].T, np.float32),
                "wqkv": wqkv_c,
                "bqkv2": np.ascontiguousarray(bqkv_c, np.float32),
                "wproj": np.ascontiguousarray(wproj_c, np.float32),
                "masks": masks,
                "ident": np.eye(P, dtype=np.float32),
            }
        )
    return in_maps


def kernel(x, Wqkv, bqkv, Wproj, bproj, _trace=False, _trace_out=None):
    from concourse.bass_utils import run_bass_kernel_spmd

    if "nc" not in _CACHE:
        _CACHE["nc"] = _build()
    nc = _CACHE["nc"]

    x = np.asarray(x, np.float32)
    Wqkv = np.asarray(Wqkv, np.float32)
    bqkv = np.asarray(bqkv, np.float32)
    Wproj = np.asarray(Wproj, np.float32)
    bproj = np.asarray(bproj, np.float32)

    in_maps = _prep_inputs(x, Wqkv, bqkv, Wproj)
    res = run_bass_kernel_spmd(
        nc, in_maps, core_ids=list(range(N_CORES)), trace=_trace
    )
    if _trace_out is not None:
        _trace_out.append(res)

    # v bias never goes to the device: attention weights sum to 1, so
    # y = y_nobias + bv, and bv @ Wproj folds into the output bias.
    bproj_eff = bproj + bqkv[2 * D : 3 * D] @ Wproj

    out = np.empty((B, T, D), np.float32)
    for b in range(B):
        acc = res.results[HPC * b]["out"].astype(np.float32)
        for g in range(1, HPC):
            acc = acc + res.results[HPC * b + g]["out"]
        out[b] = acc + bproj_eff[None, :]
    return out


# revision 43
# speedup vs baseline: 1.1413x; 1.1413x over previous
"""Causal self-attention (B=2, T=2048, D=1024, H=16, hd=64) on 8 TRN2 cores.

Sharding: 2 batches x 4 head-groups (4 heads each). Each core computes the
full pipeline for its (batch, head-group): qkv projection (transposed
layout), causal attention, and its partial output projection. The host sums
the 4 per-batch partials (tensor-parallel reduce) and adds bproj.

Device-side layout notes:
 - x is passed pre-transposed (xT [D, T]) so the qkv projection can contract
   over D on the partition dimension.
 - Scores are computed transposed (St = k @ qT, [k_tok, q_tok]) so softmax's
   exp feeds straight into att@v as the moving operand without transposes.
 - St contracts the FULL 128 partitions: stationary [kA;kB], moving the
   zero-padded per-slice q buffers [qA;0] / [0;qB]. 64-contract
   tile_position f32r matmuls run at 2 cyc/col on HW; full-contract runs
   at 1 cyc/col, so the zero-padding halves St time outright.
 - Softmax has no max-subtraction (scores are O(6) here, exp is safe) and the
   denominator is produced by augmenting v with a ones column (M=65 matmul).
 - The 1/sqrt(hd) scale is folded into Wq/bq on the host. The k bias is
   dropped (q.bk is constant along each softmax row -> invariant); the v
   bias is folded into bproj on the host (attention weights sum to 1), so
   k/v PSUM exits are plain ACT copies, off DVE.
 - exp output, v (vnat) and the causal masks are bf16: same engine rates,
   half the SBUF, and ~0.4% relative error (gate is 2e-2).
 - Softmax reciprocal is DVE reciprocal_approx_fast (keeps the recip off
   the ACT exp FIFO, which otherwise starves St PSUM slots at slice
   boundaries); the partition-crossing rec broadcast is 2 small SBUF DMAs
   split across both HWDGE rings.
 - Head: w / xt0 / xt1 DMAs are interleaved per k-tile across BOTH HWDGE
   rings (sync + scalar) and slice-0's qkv runs k-outer so the PE works at
   DMA arrival rate from ~13us instead of idling ~22us behind one FIFO.
 - The PE HAM clock gate (cold = 1.2 GHz, warm = 2.4 GHz after ~3.4us of
   sustained activity) is the reason for all the density games: paced
   filler quota in pump(), atomic qkv fill groups (short PSUM slot holds),
   and scratch warm-keeper matmuls in the final drain while the last
   recip/norm chains run.
 - proj contracts 128 (two heads stacked); odd heads reach partitions 64:128
   of the stacked y via a small SBUF->SBUF DMA (DVE cannot shift partitions).
"""

import sys

sys.path.insert(0, "/opt/trn_rl_repo")

import numpy as np
from collections import deque

B, T, D = 2, 2048, 1024
N_HEAD = 16
HD = 64  # head dim
HPC = 4  # heads per core
N_CORES = 8

P = 128
NJ = 512  # q-slice width
JT = T // NJ  # 4 q-slices
KT = D // P  # 8 contraction tiles for qkv
MT = 6  # qkv m-tiles: 2 q, 2 k, 2 v (128 dims each)
NQKV = MT * P  # 768
IT = T // P  # 16 k-token tiles

_CACHE = {}
DEBUG = False


def _build():
    import concourse.bass as bass  # noqa: F401
    import concourse.mybir as mybir
    from concourse.ap import AP
    import concourse.tile as tile
    from concourse import bacc

    F32 = mybir.dt.float32
    F32R = mybir.dt.float32r
    BF16 = mybir.dt.bfloat16
    AF = mybir.ActivationFunctionType

    class _Bacc(bacc.Bacc):
        def insert_act_table_loads(self):
            # Force every activation to resolve to the one table set that
            # holds Exp so there is a single table load at kernel start.
            from concourse.hw_specs import get_activation_tables
            import bass_rust as _br

            AF2 = mybir.ActivationFunctionType
            has_activation = any(
                isinstance(i, mybir.InstActivation)
                for b in self.main_func.blocks
                for i in b.instructions
            )
            if not has_activation:
                return
            tables = []
            for name, fns in get_activation_tables(self.m.arch).items():
                if name != "natural_log_exp_and_others":
                    fns = fns - {AF2.Exp, AF2.Ln}
                tables.append((name, fns))
            _br.insert_act_table_loads(self, tables)

    nc = _Bacc(None, target_bir_lowering=False)
    xT_d = nc.dram_tensor("xT", [D, T], F32R, kind="ExternalInput")
    wqkv_d = nc.dram_tensor("wqkv", [D, NQKV], F32R, kind="ExternalInput")
    bqkv_d = nc.dram_tensor("bqkv2", [P, MT], F32, kind="ExternalInput")
    wproj_d = nc.dram_tensor("wproj", [P, 2 * D], F32R, kind="ExternalInput")
    masks_d = nc.dram_tensor("masks", [P, P], BF16, kind="ExternalInput")
    ident_d = nc.dram_tensor("ident", [P, P], F32R, kind="ExternalInput")
    out_d = nc.dram_tensor("out", [T, D], F32, kind="ExternalOutput")

    with tile.TileContext(nc) as tc:
        with (
            tc.tile_pool(name="const", bufs=1) as const,
            tc.tile_pool(name="xp", bufs=2) as xp,
            tc.tile_pool(name="qzp", bufs=2) as qzp,
            tc.tile_pool(name="stps", bufs=4, space="PSUM") as stps,
            tc.tile_pool(name="yps", bufs=2, space="PSUM") as yps,
            tc.tile_pool(name="expp", bufs=10) as expp,
            tc.tile_pool(name="recp", bufs=2) as recp,
            tc.tile_pool(name="bcp", bufs=2) as bcp,
            tc.tile_pool(name="outp", bufs=2) as outp,
        ):
            # Per-k weight tiles so a consumer of k-tile k depends only on
            # its own DMA (not the whole-weight load).
            w_k = [
                const.tile([P, NQKV], F32R, tag=f"wk{k}", name=f"wk{k}")
                for k in range(KT)
            ]
            bias_sb = const.tile([P, MT], F32)
            wp_sb = const.tile([P, 2, D], F32R)
            masks_sb = const.tile([P, P], BF16)
            ident = const.tile([P, P], F32R)
            # k (slots 0,1 = head-pairs) and v (slots 2,3) in transposed
            # layout. q lives in per-slice zero-padded buffers (qz below):
            # St contracts the FULL 128 partitions ([kA;kB] stationary
            # against [qA;0] / [0;qB] moving) because 64-contract
            # tile_position matmuls run at half rate on f32r.
            qkvT_sb = const.tile([P, 4, T], F32R)
            vnat_sb = const.tile([P, 2, IT, 192], BF16)
            yt2_sb = const.tile([P, 2, T], F32R)

            w_r = wqkv_d.rearrange("(kt p) n -> p kt n", p=P)
            xT_r = xT_d.rearrange("(kt p) t -> p kt t", p=P)

            xts = {}

            def emit_xt(j, eng=None):
                eng = eng or nc.sync
                tl = []
                for k in range(KT):
                    t = xp.tile([P, NJ], F32R, tag=f"xt{k}", name=f"xt{j}_{k}")
                    eng.dma_start(t[:], xT_r[:, k, j * NJ : (j + 1) * NJ])
                    tl.append(t)
                xts[j] = tl

            # ---- Head: interleaved dual-ring DMAs -----------------------
            # sync ring: xt0 k-tiles then wproj; scalar ring: w k-tiles then
            # bias/masks/ident. Paired k emission means k-tile pairs arrive
            # in ascending-k order at roughly HBM rate on both rings.
            xt0 = []
            for k in range(KT):
                t = xp.tile([P, NJ], F32R, tag=f"xt{k}", name=f"xt0_{k}")
                nc.sync.dma_start(t[:], xT_r[:, k, 0:NJ])
                xt0.append(t)
                nc.scalar.dma_start(w_k[k][:], w_r[:, k, :])
            xts[0] = xt0
            nc.sync.dma_start(bias_sb[:], bqkv_d[:])
            nc.sync.dma_start(masks_sb[:], masks_d[:])
            nc.sync.dma_start(ident[:], ident_d[:])
            # slice-1 x tiles + wproj go on the sync ring: anything queued
            # on the scalar ring blocks the ACT exp stream behind its
            # backpressured DMA issues.
            xt1 = []
            for k in range(KT):
                t = xp.tile([P, NJ], F32R, tag=f"xt{k}", name=f"xt1_{k}")
                nc.sync.dma_start(t[:], xT_r[:, k, NJ : 2 * NJ])
                xt1.append(t)
            xts[1] = xt1
            nc.sync.dma_start(
                wp_sb[:], wproj_d.rearrange("p (g d) -> p g d", g=2)
            )

            for h2 in range(2):
                nc.gpsimd.memset(vnat_sb[:, h2, :, :], 1.0)

            # ---- Slice-0 qkv: k-outer so the PE tracks DMA arrival ------
            # 3 m-pair accumulators (q, k, v pairs): 2 from the st pool, 1
            # borrowed from the y pool (attention has not started yet).
            hacc = [
                stps.tile([P, NJ], F32, tag="st", name="q0_m0"),
                stps.tile([P, NJ], F32, tag="st", name="q0_m1"),
                stps.tile([P, NJ], F32, tag="st", name="q0_m2"),
                stps.tile([P, NJ], F32, tag="st", name="q0_m3"),
                yps.tile([P, NJ], F32, tag="y", name="q0_m4"),
                yps.tile([P, NJ], F32, tag="y", name="q0_m5"),
            ]
            for k in range(KT - 1):
                for m in (2, 3, 0, 1, 4, 5):
                    nc.tensor.matmul(
                        hacc[m][:],
                        w_k[k][:, m * P : (m + 1) * P],
                        xts[0][k][:],
                        start=(k == 0),
                        stop=False,
                    )

            def _head_last_k(mp):
                for g in range(2):
                    m = 2 * mp + g
                    nc.tensor.matmul(
                        hacc[m][:],
                        w_k[KT - 1][:, m * P : (m + 1) * P],
                        xts[0][KT - 1][:],
                        start=False,
                        stop=True,
                    )
            # Per-slice zero-padded q buffers: qz[(j, hp)] holds
            # [qA; 0] (slot 0) and [0; qB] (slot 1) for head-pair hp.
            qz_map = {}

            def alloc_qz(j, hp):
                t = qzp.tile([P, 2, NJ], F32R, tag=f"qz{hp}", name=f"qz{j}_{hp}")
                nc.gpsimd.memset(t[:, :, :].bitcast(F32), 0.0)
                qz_map[(j, hp)] = t
                return t

            def emit_q_adds(j, hp, ps, on_act=False):
                # ps: [P, NJ]-shaped PSUM AP holding m-tile hp of q.
                # Fills run these on ACT (Identity takes a [P,1] bias AP and
                # shares the exp table set): at boundaries ACT is starved
                # waiting for the next St, so the adds clear immediately
                # instead of queueing in the bursty boundary DVE window.
                t = qz_map[(j, hp)]
                nc.vector.tensor_scalar_add(
                    t[0:HD, 0, :], ps[0:HD], bias_sb[0:HD, hp : hp + 1]
                )
                nc.vector.tensor_scalar_add(
                    t[HD:P, 1, :], ps[HD:P], bias_sb[HD:P, hp : hp + 1]
                )

            # exits: k pair first (St needs k tiles), then q, then v.
            # k/v exits are plain copies on ACT (k bias is softmax-
            # invariant; v bias is folded into bproj on the host).
            _head_last_k(1)
            for g in range(2):
                nc.scalar.activation(
                    qkvT_sb[:, g, 0:NJ], hacc[2 + g][:], AF.Copy
                )
            _head_last_k(0)
            for hp in range(2):
                alloc_qz(0, hp)
                emit_q_adds(0, hp, hacc[hp][:])
            _head_last_k(2)
            for g in range(2):
                nc.scalar.activation(
                    qkvT_sb[:, 2 + g, 0:NJ], hacc[4 + g][:], AF.Copy
                )

            # ---- Stage-2 fills for later slices (schedulable units) -----

            def emit_qkv_m(j, m):
                # one atomic fill: 8 accumulating matmuls + the PSUM exit,
                # so the st-pool slot is held for only ~1 unit
                ps = stps.tile([P, NJ], F32, tag="st", name=f"qkvps{j}_{m}")
                if m < 2:
                    alloc_qz(j, m)
                for k in range(KT):
                    nc.tensor.matmul(
                        ps[:],
                        w_k[k][:, m * P : (m + 1) * P],
                        xts[j][k][:],
                        start=(k == 0),
                        stop=(k == KT - 1),
                    )
                if m < 2:
                    emit_q_adds(j, m, ps[:], on_act=True)
                else:
                    nc.scalar.activation(
                        qkvT_sb[:, m - 2, j * NJ : (j + 1) * NJ],
                        ps[:],
                        AF.Copy,
                    )

            def emit_vt(h2, ii):
                pt = stps.tile([P, NJ], F32R, tag="st", name=f"vt{h2}_{ii}")
                nc.tensor.transpose(
                    pt[:, 0:P], qkvT_sb[:, 2 + h2, ii * P : (ii + 1) * P], ident[:]
                )
                nc.vector.tensor_copy(vnat_sb[:, h2, ii, 0:HD], pt[:, 0:HD])
                nc.vector.tensor_copy(
                    vnat_sb[:, h2, ii, 2 * HD : 2 * HD + P - HD], pt[:, HD:P]
                )

            # slice-0 v transposes
            for h2 in range(2):
                for ii in range(4):
                    emit_vt(h2, ii)


            fill_q = deque()

            def push_fill(j):
                for m in range(MT):
                    fill_q.append(("qkv", j, m))
                for h2 in range(2):
                    for ii in range(4 * j, 4 * j + 4):
                        fill_q.append(("vt", j, h2, ii))

            def emit_fill(item):
                if item[0] == "qkv":
                    emit_qkv_m(item[1], item[2])
                else:
                    emit_vt(item[2], item[3])

            # pre-emit a few slice-1 qkv fills: their matmuls cover the PE
            # during slice 0's exp-bound St ramp (xt1 landed in the head)
            push_fill(1)
            for _ in range(3):
                emit_fill(fill_q.popleft())

            # ---- Stage 3: attention per head-pair ----------------------
            # Software-pipelined: St(i) is issued before Y(i-1) so ACT's
            # exp(i-1) overlaps the PE's St(i); normalization of slice (hp,j)
            # is deferred into slice (hp,j)+1's loop so the reciprocal's
            # latency hides behind matmul work.

            def emit_recip(pend):
                # 1/d on DVE (reciprocal_approx_fast, ~18 bits): keeps the
                # reciprocal off the ACT exp stream, whose FIFO backlog was
                # starving the St PSUM slots at every slice boundary. The Y
                # matmuls deliver d pre-broadcast: par0's on partitions
                # 64:128, par1's on 0:64. A small SBUF->SBUF DMA moves each
                # reciprocal to the partition range its y values occupy
                # (engines cannot cross partitions; DMA can).
                hp_, j_, y2_ = pend
                recb = recp.tile([P, 2, NJ], F32, tag="recb")
                # full-tile recip: the y quadrants produce junk in slots the
                # DMAs below never read; one wide DVE op per stage beats two
                # narrow ones on per-op overhead.
                nc.vector.reciprocal_approx_fast(recb[:, :, :], y2_[:, :, :])
                rec2 = bcp.tile([P, 2, NJ], F32, tag="bc")
                nc.sync.dma_start(rec2[0:HD, 0, :], recb[64:128, 0, :])
                nc.scalar.dma_start(rec2[64:128, 1, :], recb[0:HD, 1, :])
                return (hp_, j_, y2_, rec2)

            def emit_norm(pend):
                hp_, j_, y2_, rec2 = pend
                # head 2*hp_ -> partitions 0:64; head 2*hp_+1 -> 64:128
                nc.vector.tensor_mul(
                    yt2_sb[0:HD, hp_, j_ * NJ : (j_ + 1) * NJ],
                    y2_[0:HD, 0, :],
                    rec2[0:HD, 0, :],
                )
                nc.vector.tensor_mul(
                    yt2_sb[HD:P, hp_, j_ * NJ : (j_ + 1) * NJ],
                    y2_[HD:P, 1, :],
                    rec2[HD:P, 1, :],
                )

            def emit_y(rec):
                hp_, pi, pexp, y2_, last, j_ = rec
                r_ = pi - 4 * j_
                # f32r matmuls below 256 moving cols run at 1/4 rate, so
                # clamp the diag-block start column to keep >=256 width; the
                # extra columns read memset zeros from the exp tile.
                cy = min(P * r_, NJ - 2 * P) if r_ > 0 else 0
                # weight layout per k-tile: [vA(0:64) | ones(64:128) | vB
                # (128:192)]. par0 uses cols 0:128 -> yA on partitions 0:64,
                # denominator replicated on 64:128. par1 uses cols 64:192 ->
                # denominator on 0:64, yB on 64:128 (where the stacked proj
                # layout wants it -- no partition shift needed).
                nc.tensor.matmul(
                    y2_[:, 0, cy:NJ],
                    vnat_sb[:, hp_, pi, 0:P],
                    pexp[:, 0, cy:NJ],
                    start=(pi == 0),
                    stop=last,
                )
                nc.tensor.matmul(
                    y2_[:, 1, cy:NJ],
                    vnat_sb[:, hp_, pi, HD : HD + P],
                    pexp[:, 1, cy:NJ],
                    start=(pi == 0),
                    stop=last,
                )

            drain_mode = [False]

            def emit_proj(item):
                qm_i, n = item
                po = stps.tile([P, NJ], F32, tag="st")
                for g in range(2):
                    nc.tensor.matmul(
                        po[:],
                        yt2_sb[:, g, qm_i * P : (qm_i + 1) * P],
                        wp_sb[:, g, n * NJ : (n + 1) * NJ],
                        start=(g == 0),
                        stop=(g == 1),
                    )
                ot = outp.tile([P, NJ], F32)
                # in the drain ACT is idle: alternate the PSUM exit between
                # DVE and ACT so proj items don't serialize on one copy engine
                if drain_mode[0] and (qm_i + n) % 2 == 1:
                    nc.scalar.activation(ot[:], po[:], AF.Copy)
                else:
                    nc.vector.tensor_copy(ot[:], po[:])
                nc.sync.dma_start(
                    out_d[qm_i * P : (qm_i + 1) * P, n * NJ : (n + 1) * NJ], ot[:]
                )

            # Global software pipeline over slices (j outer, hp inner):
            # y matmuls trail the St/exp stream by DEPTH iterations and spill
            # across slice boundaries; each slice's normalization runs inside
            # a later slice's loop; proj groups for q-slice j are spread one
            # per iteration once both head-pairs of j are normalized.
            DEPTH = 6
            y_q = deque()  # (hp, i, exp2, y2holder, last, j)
            norm_q = deque()  # (hp, j, y2, rec)
            proj_items = deque()  # (qm_i, n)
            y2_map = {}  # (hp, j) -> y2 tile, allocated lazily at first y

            def emit_y2(rec):
                hp_, pi, pexp, _, last, j_ = rec
                if pi == 0:
                    y2_map[(hp_, j_)] = yps.tile(
                        [P, 2, NJ], F32, tag="y", name=f"y2_{hp_}_{j_}"
                    )
                emit_y((hp_, pi, pexp, y2_map[(hp_, j_)], last, j_))
                if last:
                    norm_q.append(
                        emit_recip((hp_, j_, y2_map.pop((hp_, j_))))
                    )

            calls_left = [1]

            def pump(i):
                # pace PE filler work (qkv/vt fills + proj) evenly across the
                # slice's pump calls so every ACT-bound unit gets PE cover and
                # the DVE bias-adds land away from the boundary burst
                if i >= 2 and norm_q:
                    pend = norm_q.popleft()
                    emit_norm(pend)
                    if pend[0] == 1:  # second head-pair of slice j done
                        for qq in range(4 * pend[1], 4 * pend[1] + 4):
                            proj_items.append((qq, 0))
                            proj_items.append((qq, 1))
                avail = len(fill_q) + (len(proj_items) if i >= 2 else 0)
                quota = -(-avail // max(1, calls_left[0]))
                for _ in range(min(quota, 2)):
                    if fill_q:
                        emit_fill(fill_q.popleft())
                    elif i >= 2 and proj_items:
                        emit_proj(proj_items.popleft())
                calls_left[0] = max(1, calls_left[0] - 1)

            for j in range(JT):
                if j + 2 < JT:
                    emit_xt(j + 2)
                if 0 < j and j + 1 < JT:
                    push_fill(j + 1)
                n_i = 4 * j + 4
                calls_left[0] = 2 * n_i
                for i in range(n_i):
                    for hp in range(2):
                        if len(y_q) > DEPTH:
                            emit_y2(y_q.popleft())
                        # per-par 1-bank St tiles: 4 independent slots
                        # decouple the St stream from exp at double depth
                        # (fewer PE stalls -> HAM stays warm)
                        st_a = stps.tile([P, NJ], F32, tag="st")
                        st_b = stps.tile([P, NJ], F32, tag="st")
                        r = i - 4 * j
                        c0 = P * r if r > 0 else 0
                        # St matmuls also keep >=256 moving width (the extra
                        # columns land in the masked region the exp memset
                        # overwrites).
                        cs = min(c0, NJ - 2 * P)
                        # full-128 contraction: stationary [kA;kB], moving
                        # the zero-padded [qA;0] / [0;qB] buffers. One
                        # LDWEIGHTS serves both head outputs.
                        kst = qkvT_sb[:, hp, i * P : (i + 1) * P]
                        qzj = qz_map[(j, hp)]
                        nc.tensor.matmul(
                            st_a[:, cs:NJ],
                            kst,
                            qzj[:, 0, cs:NJ],
                            start=True,
                            stop=True,
                        )
                        nc.tensor.matmul(
                            st_b[:, cs:NJ],
                            kst,
                            qzj[:, 1, cs:NJ],
                            start=True,
                            stop=True,
                        )
                        exp2 = expp.tile([P, 2, NJ], BF16, tag="exp")
                        if c0 > 0:
                            nc.vector.memset(exp2[:, :, 0:c0], 0.0)
                        nc.scalar.activation(
                            exp2[:, 0, c0:NJ], st_a[:, c0:NJ], AF.Exp
                        )
                        nc.scalar.activation(
                            exp2[:, 1, c0:NJ], st_b[:, c0:NJ], AF.Exp
                        )
                        if r >= 0:
                            m_ap = masks_sb[:, 0:P]
                            m_bc = AP(
                                m_ap.tensor,
                                m_ap.offset,
                                [list(m_ap.ap[0]), [0, 2], list(m_ap.ap[1])],
                            )
                            nc.vector.tensor_mul(
                                exp2[:, :, c0 : c0 + P],
                                exp2[:, :, c0 : c0 + P],
                                m_bc,
                            )
                        y_q.append((hp, i, exp2, None, i == n_i - 1, j))
                        pump(i)
                # slice j+1's qkv must be complete before its St reads;
                # vt items may ride over the boundary (Y consumes the new
                # v tiles only DEPTH units into the next slice, and the
                # tile framework orders the writes regardless)
                while fill_q and fill_q[0][0] == "qkv":
                    emit_fill(fill_q.popleft())

            # drain hp0's Y stream first so its recip/norm/proj overlap
            # hp1's remaining Y matmuls
            for rec in sorted(y_q, key=lambda r: (r[0], r[1])):
                emit_y2(rec)
            y_q.clear()

            def emit_warmkeep(n):
                # scratch matmuls that keep the PE activity monitor hot
                # while the recip/norm chains run, so the final projs
                # execute at full clock instead of half
                for _ in range(n):
                    wk = stps.tile([P, NJ], F32, tag="st", name="warm")
                    nc.tensor.matmul(
                        wk[:], w_k[0][:, 0:P], xts[JT - 1][0][:],
                        start=True, stop=True,
                    )

            drain_mode[0] = True
            emit_warmkeep(3)
            while norm_q:
                pend = norm_q.popleft()
                emit_norm(pend)
                if pend[0] == 1:
                    for qq in range(4 * pend[1], 4 * pend[1] + 4):
                        proj_items.append((qq, 0))
                        proj_items.append((qq, 1))
                emit_warmkeep(3)
            while proj_items:
                emit_proj(proj_items.popleft())

    nc.compile()
    return nc


def _prep_inputs(x, Wqkv, bqkv, Wproj):
    """Per-core input maps. Core c -> batch c//4, heads 4*(c%4) .. +4."""
    import ml_dtypes

    scale = np.float32(1.0 / np.sqrt(HD))
    pp = np.arange(P)[:, None]
    ff = np.arange(P)[None, :]
    masks = (ff >= pp).astype(ml_dtypes.bfloat16)

    in_maps = []
    for c in range(N_CORES):
        b, g = divmod(c, HPC)
        cs = slice(256 * g, 256 * g + 256)
        wq = Wqkv[:, 0 * D :][:, cs] * scale
        wk = Wqkv[:, 1 * D : 2 * D][:, cs]
        wv = Wqkv[:, 2 * D : 3 * D][:, cs]
        wqkv_c = np.ascontiguousarray(np.concatenate([wq, wk, wv], axis=1), np.float32)
        bq = bqkv[0 * D :][cs] * scale
        bk = bqkv[1 * D : 2 * D][cs]
        bv = bqkv[2 * D : 3 * D][cs]
        bqkv_c = np.concatenate([bq, bk, bv]).reshape(MT, P).T
        # head-pairs stacked on partitions: row p, pair g2, col d ->
        # Wproj[256*g + 128*g2 + p, d]
        wproj_c = (
            Wproj[256 * g : 256 * (g + 1), :]
            .reshape(2, P, D)
            .transpose(1, 0, 2)
            .reshape(P, 2 * D)
        )
        in_maps.append(
            {
                "xT": np.ascontiguousarray(x[b].T, np.float32),
                "wqkv": wqkv_c,
                "bqkv2": np.ascontiguousarray(bqkv_c, np.float32),
                "wproj": np.ascontiguousarray(wproj_c, np.float32),
                "masks": masks,
                "ident": np.eye(P, dtype=np.float32),
            }
        )
    return in_maps


def kernel(x, Wqkv, bqkv, Wproj, bproj, _trace=False, _trace_out=None):
    from concourse.bass_utils import run_bass_kernel_spmd

    if "nc" not in _CACHE:
        _CACHE["nc"] = _build()
    nc = _CACHE["nc"]

    x = np.asarray(x, np.float32)
    Wqkv = np.asarray(Wqkv, np.float32)
    bqkv = np.asarray(bqkv, np.float32)
    Wproj = np.asarray(Wproj, np.float32)
    bproj = np.asarray(bproj, np.float32)

    in_maps = _prep_inputs(x, Wqkv, bqkv, Wproj)
    res = run_bass_kernel_spmd(
        nc, in_maps, core_ids=list(range(N_CORES)), trace=_trace
    )
    if _trace_out is not None:
        _trace_out.append(res)

    # v bias never goes to the device: attention weights sum to 1, so
    # y = y_nobias + bv, and bv @ Wproj folds into the output bias.
    bproj_eff = bproj + bqkv[2 * D : 3 * D] @ Wproj

    out = np.empty((B, T, D), np.float32)
    for b in range(B):
        acc = res.results[HPC * b]["out"].astype(np.float32)
        for g in range(1, HPC):
            acc = acc + res.results[HPC * b + g]["out"]
        out[b] = acc + bproj_eff[None, :]
    return out
